# revision 1
# baseline (speedup 1.0000x reference)
"""Kernel for nn_DSRB: spiking dense-CNN block, data-parallel on Trainium.

Strategy: data-parallel over the batch axis B=4 across NeuronCores via
jax.pmap. Everything in the network is per-batch-element independent except
the training-mode BatchNorm statistics, which are all-reduced with
jax.lax.psum. The LIF recurrence runs over T=4 locally per device.
"""

import numpy as np
import jax
import jax.numpy as jnp
from functools import partial

TAU = 2.0
VTH = 0.15
EPS = 1e-5

T, B, C, H, W = 4, 4, 64, 128, 128


def _spike(x):
    return (x >= 0.0).astype(x.dtype)


def _lif(xseq):
    v0 = jnp.zeros_like(xseq[0])

    def step(v, xt):
        v = v * (1.0 - 1.0 / TAU) + xt
        s = _spike(v - VTH)
        return v * (1.0 - s), s

    _, spikes = jax.lax.scan(step, v0, xseq)
    return spikes


def _conv2d(x, w, pad):
    # conv as 9 shifted matmuls (dot_general) — the neuron compiler's
    # TransformConvOp pass is broken in this toolchain.
    kh, kw = w.shape[2], w.shape[3]
    if kh == 1 and kw == 1:
        return jnp.einsum('oi,nihw->nohw', w[:, :, 0, 0], x,
                          preferred_element_type=jnp.float32)
    n, ci, hh, ww = x.shape
    xp = jnp.pad(x, ((0, 0), (0, 0), (pad, pad), (pad, pad)))
    y = None
    for dy in range(kh):
        for dx in range(kw):
            xs = jax.lax.dynamic_slice(xp, (0, 0, dy, dx), (n, ci, hh, ww))
            t = jnp.einsum('oi,nihw->nohw', w[:, :, dy, dx], xs,
                           preferred_element_type=jnp.float32)
            y = t if y is None else y + t
    return y


def _bn_psum(x, g, b, axis_name):
    # x: [T*Bl, C, H, W] local shard; stats all-reduced over the batch axis
    n_dev = jax.lax.psum(1, axis_name)
    m = jax.lax.psum(x.mean((0, 2, 3)), axis_name) / n_dev
    m2 = jax.lax.psum((x * x).mean((0, 2, 3)), axis_name) / n_dev
    v = m2 - m * m
    scale = g * jax.lax.rsqrt(v + EPS)
    return (x - m[:, None, None]) * scale[:, None, None] + b[:, None, None]


def _block(x, w0, w1, w2, w3, g0, g1, g2, g3, b0, b1, b2, b3,
           lff_w, t_w, t_b, c_w1, c_b1, c_w2, c_b2, s_w, s_b):
    # x: [T, Bl, C, H, W] local shard (Bl = 1)
    Tl, Bl = x.shape[0], x.shape[1]
    feats = x
    for w, g, bb in zip((w0, w1, w2, w3), (g0, g1, g2, g3), (b0, b1, b2, b3)):
        s = _lif(feats).reshape(Tl * Bl, feats.shape[2], H, W)
        y = _bn_psum(_conv2d(s, w, 1), g, bb, 'b').reshape(Tl, Bl, -1, H, W)
        feats = jnp.concatenate([feats, y], axis=2)
    s = _lif(feats).reshape(Tl * Bl, feats.shape[2], H, W)
    out = _conv2d(s, lff_w, 0).reshape(Tl, Bl, C, H, W)

    # attention — fully local per batch element
    xp = jnp.transpose(out, (1, 2, 0, 3, 4))  # [Bl,C,T,H,W]
    temp = jax.nn.sigmoid(t_w * xp.mean((1, 2, 3, 4)) + t_b)  # [Bl]
    xt = xp * temp[:, None, None, None, None]
    pooled = xt.mean((2, 3, 4))  # [Bl,C]
    h = jax.nn.relu(pooled @ c_w1.T + c_b1)
    ca = jax.nn.sigmoid(h @ c_w2.T + c_b2)
    xc = xt * ca[:, :, None, None, None]
    sp = xc.mean(1).reshape(Bl * Tl, 1, H, W)
    sa = jax.nn.sigmoid(_conv2d(sp, s_w, 1) + s_b).reshape(Bl, Tl, H, W)
    xs = xc * sa[:, None]
    # return only the (small-magnitude) attention term, in bf16, to halve
    # the device->host transfer; the +x residual is added on host in fp32.
    return jnp.transpose(xs, (2, 0, 1, 3, 4)).astype(jnp.bfloat16)


_pblock = None


def _get_pblock():
    global _pblock
    if _pblock is None:
        _pblock = jax.pmap(_block, axis_name='b',
                           in_axes=(1,) + (None,) * 21,
                           out_axes=1, devices=jax.devices()[:B])
    return _pblock


def kernel(**inputs):
    # feed numpy directly: pmap transfers each batch shard straight to its
    # device instead of staging the full array on device 0 first.
    x = np.ascontiguousarray(np.asarray(inputs['x'], np.float32))
    args = []
    for name in ('w0', 'w1', 'w2', 'w3', 'g0', 'g1', 'g2', 'g3',
                 'b0', 'b1', 'b2', 'b3', 'lff_w', 't_w', 't_b',
                 'c_w1', 'c_b1', 'c_w2', 'c_b2', 's_w', 's_b'):
        args.append(np.asarray(inputs[name], np.float32))
    # reshape to [T, B, 1, C, H, W] so each device gets Bl=1
    xs = x.reshape(T, B, 1, C, H, W)
    out = _get_pblock()(xs, *args)  # [T, B, 1, C, H, W] bf16 (xs term only)
    res = np.asarray(out).astype(np.float32).reshape(T, B, C, H, W)
    res += x
    return res

